# revision 16
# baseline (speedup 1.0000x reference)
"""ClusterNet (vq_codebook) kernel for 8x Trainium2 NeuronCores (Bass/Tile).

Reference math (ALPHA = 1):
    d2   = |z - c|^2                     z: (8192, 2048)  c: (512, 2048)
    Qun  = (1 + sqrt(d2))^-1
    Q    = Qun / rowsum(Qun)
    P    = (Q^2 / colsum(Q)) / rowsum(Q^2 / colsum(Q))
    out  = stack([Q, P])                 (2, 8192, 512) float32

Distribution: data-parallel over the batch — 1024 rows per core, centroids
replicated. Cross-core communication is an AllGather of the per-core
column-sum of Q (512 floats); each core then reduces + broadcasts the 8
partials with a single K=8 matmul.

Per-core pipeline (8 m-tiles of 128 rows):
  PE   : dummy warm-up matmuls first (keep HAM un-throttled through the
         input-DMA window), then PSUM accumulates d2 - 1 per tile: one K=2
         f32r affine matmul (rows csq-1 / zsq against ones) + 8 fp8e4
         DoubleRow matmuls (K=256 each) carrying the -2*z.c cross term.
         Tiles 0-3 stream k-outer so matmuls start as soon as the first
         input chunk lands; tiles 4-7 run m-outer so their results pipeline
         into the tail.
  ACT  : sim = Sqrt(psum + 1) = sqrt(d2)  (table preloaded at t=0).
  DVE  : r = 1/(d2-1) (approx_fast); qun = (sim - 1) * r  [since
         1/(1+s) = (s-1)/(d2-1)] with fused row-sum; rq = 1/rowsum
         (f32 + bf16 copies of rq and qun for the matvec).
  PE   : per-tile weighted matvec (lhsT=rq_bf, rhs=qun_bf) accumulates the
         local colsum of Q in PSUM — so neither the Q scale nor the Q
         store sits on the collective trigger path.
  CC   : AllGather of the [1,512] partial (a dummy warm-up AllGather is
         triggered at kernel start so ncfw is awake and launch skew is
         absorbed off the critical path).
  under the collective window:
         Q = qun * rq (DVE tensor_scalar, bf16 out) + store;
         q2 = qun^2 (ACT Square, f32).
  PE   : ones[8,128]^T @ gathered[8,512] = colsum broadcast to 128
         partitions in one matmul; DVE reciprocal -> 1/s.
  DVE  : W = q2 * (1/s) with fused row-accumulate; P = W * (1/rowsumW)
         split between DVE and ACT, written bf16.

Input DMA is interleaved across the sync and scalar queues with both g=0
chunks first, so the first DoubleRow group lands early. Host prepares fp8
transposed operands and exact f32 squared norms; Q/P come back bf16 and
are upcast to f32 on the host.
"""

import sys

import numpy as np

if "/opt/trn_rl_repo" not in sys.path:
    sys.path.insert(0, "/opt/trn_rl_repo")

import ml_dtypes

import concourse.bacc as bacc
import concourse.mybir as mybir
import concourse.tile as tile
from concourse.bass_utils import run_bass_kernel_spmd

F8 = ml_dtypes.float8_e4m3
BF16 = ml_dtypes.bfloat16

N_CORES = 8
BS, NH, NCL = 8192, 2048, 512
B_CORE = BS // N_CORES          # 1024 rows per core
M_TILES = B_CORE // 128         # 8
G = NH // 256                   # 8 DoubleRow groups (256 contraction rows each)

_nc_cache = None


def _build_nc():
    F = mybir.ActivationFunctionType
    A = mybir.AluOpType
    f32 = mybir.dt.float32
    f32r = mybir.dt.float32r
    bf16 = mybir.dt.bfloat16
    f8 = mybir.dt.float8e4
    DR = mybir.MatmulPerfMode.DoubleRow

    nc = bacc.Bacc("TRN2", target_bir_lowering=False, debug=False,
                   num_devices=N_CORES)
    zt_d = nc.dram_tensor("zt", [G, 128, 2 * B_CORE], f8, kind="ExternalInput")
    ct_d = nc.dram_tensor("ct", [G, 128, 2 * NCL], f8, kind="ExternalInput")
    affl_d = nc.dram_tensor("affl", [2, B_CORE], f32r, kind="ExternalInput")
    affr_d = nc.dram_tensor("affr", [2, NCL], f32r, kind="ExternalInput")
    ones8_d = nc.dram_tensor("ones8", [8, 128], f32r, kind="ExternalInput")

    q_out = nc.dram_tensor("q", [B_CORE, NCL], bf16, kind="ExternalOutput")
    p_out = nc.dram_tensor("p", [B_CORE, NCL], bf16, kind="ExternalOutput")

    groups = [list(range(N_CORES))]

    with tile.TileContext(nc) as tc:
        with (
            tc.tile_pool(name="zin", bufs=1) as zin,
            tc.tile_pool(name="cin", bufs=1) as cin,
            tc.tile_pool(name="work", bufs=1) as work,
            tc.tile_pool(name="small", bufs=1) as small,
            tc.tile_pool(name="qout", bufs=3) as qout,
            tc.tile_pool(name="pout", bufs=3) as pout,
            tc.tile_pool(name="psum", bufs=6, space="PSUM") as psum,
            tc.tile_pool(name="cpsum", bufs=1, space="PSUM") as cpsum,
            tc.tile_pool(name="dram", bufs=1, space="DRAM") as dram,
        ):
            # --- tiny operands for PE warm-up + ACT table preload
            dml = small.tile([1, 128], bf16, tag="dml")
            nc.vector.memset(dml, 0.0)
            dmr = small.tile([1, NCL], bf16, tag="dmr")
            nc.vector.memset(dmr, 0.0)
            actp = small.tile([1, 8], f32, tag="actp")
            actq = small.tile([1, 8], f32, tag="actq")
            nc.vector.memset(actp, 1.0)

            # --- input DMA interleaved across sync+scalar queues, both g=0
            # chunks first so the PE can stream as groups arrive.
            zt = zin.tile([128, G, 2, B_CORE], f8, tag="zt")
            ct = cin.tile([128, G, 2, NCL], f8, tag="ct")
            affl = small.tile([2, B_CORE], f32r, tag="affl")
            affr = small.tile([2, NCL], f32r, tag="affr")

            def dma_ct(eng, g):
                eng.dma_start(
                    out=ct[:, g], in_=ct_d.ap()[g].rearrange(
                        "p (k n) -> p k n", k=2))

            def dma_zt(eng, g):
                eng.dma_start(
                    out=zt[:, g], in_=zt_d.ap()[g].rearrange(
                        "p (k m) -> p k m", k=2))

            dma_ct(nc.sync, 0)
            dma_zt(nc.scalar, 0)
            for g in range(1, G):
                dma_zt(nc.sync if g % 2 == 0 else nc.scalar, g)
                dma_ct(nc.scalar if g % 2 == 0 else nc.sync, g)
            nc.scalar.dma_start(out=affl, in_=affl_d.ap())
            nc.scalar.dma_start(out=affr, in_=affr_d.ap())
            nc.scalar.activation(actq, actp, F.Sqrt, bias=1.0)

            # --- workspaces
            sim_all = work.tile([128, M_TILES, NCL], f32, tag="sim")
            r_all = work.tile([128, M_TILES, NCL], f32, tag="r")
            qun_all = work.tile([128, M_TILES, NCL], f32, tag="qun")
            qbf_all = work.tile([128, M_TILES, NCL], bf16, tag="qbf")
            q2_all = work.tile([128, M_TILES, NCL], f32, tag="q2")
            w_all = work.tile([128, M_TILES, NCL], f32, tag="w")
            sq_all = small.tile([128, M_TILES], f32, tag="sq")
            rq_all = small.tile([128, M_TILES], f32, tag="rq")
            rqb_all = small.tile([128, M_TILES], bf16, tag="rqb")
            ws_all = small.tile([128, M_TILES], f32, tag="ws")
            rw_all = small.tile([128, M_TILES], f32, tag="rw")
            ones8 = small.tile([8, 128], f32r, tag="ones8")
            nc.sync.dma_start(out=ones8, in_=ones8_d.ap())
            q_all = work.tile([128, M_TILES, NCL], bf16, tag="qall")
            cs_sb = small.tile([1, NCL], f32, tag="cssb")
            ag_sb = small.tile([8, NCL], f32r, tag="agsb")
            rs_inv = small.tile([128, NCL], f32, tag="rsinv")

            warm_in = dram.tile([1, 8], f32)
            warm_out = dram.tile([8, 8], f32, addr_space="Shared")
            cc_in = dram.tile([1, NCL], f32)
            cc_out = dram.tile([8, NCL], f32, addr_space="Shared")

            # --- warm-up collective: pays the ~50us ncfw cold-start early,
            # in the background, so the real AllGather below runs on warm
            # ncfw. The payload is garbage (uninitialized DRAM) by design.
            nc.gpsimd.collective_compute(
                "AllGather", A.bypass, replica_groups=groups,
                ins=[warm_in.opt()], outs=[warm_out.opt()],
            )

            ps = [None] * M_TILES

            # --- PE warm-up: K=1 dummy matmuls bridge the input-DMA window
            # so HAM never sees an idle window before the real stream.
            dps = psum.tile([128, NCL], f32, name="dps", tag="mm")
            for _ in range(8):
                nc.tensor.matmul(dps, lhsT=dml, rhs=dmr, start=True,
                                 stop=True)

            def mm_tile(m):
                ps[m] = psum.tile([128, NCL], f32, name=f"ps{m}", tag="mm")

            def mm_group(m, g):
                ms = slice(m * 128, (m + 1) * 128)
                nc.tensor.matmul(
                    ps[m], lhsT=zt[:, g, :, ms], rhs=ct[:, g],
                    start=(g == 0), stop=False, perf_mode=DR)

            def mm_affine(m):
                ms = slice(m * 128, (m + 1) * 128)
                nc.tensor.matmul(
                    ps[m], lhsT=affl[:, ms], rhs=affr[:, :],
                    start=False, stop=True)

            def dve_tail(m):
                sim = sim_all[:, m, :]
                r = r_all[:, m, :]
                qun = qun_all[:, m, :]
                nc.vector.reciprocal_approx_fast(out=r, in_=ps[m][:, :])
                nc.vector.scalar_tensor_tensor(
                    out=qun, in0=sim, scalar=1.0, in1=r,
                    op0=A.subtract, op1=A.mult,
                    accum_out=sq_all[:, m:m + 1])
                nc.vector.reciprocal(rq_all[:, m:m + 1], sq_all[:, m:m + 1])
                nc.vector.tensor_copy(rqb_all[:, m:m + 1],
                                      rq_all[:, m:m + 1])
                nc.vector.tensor_copy(qbf_all[:, m, :], qun)

            def act_sqrt(m):
                nc.scalar.activation(sim_all[:, m, :], ps[m][:, :],
                                     F.Sqrt, bias=1.0)

            cps = cpsum.tile([1, NCL], f32, tag="cs")

            def matvec(m, start, stop):
                nc.tensor.matmul(
                    cps, lhsT=rqb_all[:, m:m + 1], rhs=qbf_all[:, m, :],
                    start=start, stop=stop)

            # wave A: tiles 0-3, k-outer (stream groups as they arrive)
            for m in range(4):
                mm_tile(m)
            for g in range(G):
                for m in range(4):
                    mm_group(m, g)
            for m in range(4):
                mm_affine(m)
            for m in range(4):
                act_sqrt(m)
            for m in range(4):
                dve_tail(m)

            # wave B: tiles 4-7, m-outer; wave-A matvecs ride between tiles
            for m in range(4, M_TILES):
                mm_tile(m)
                for g in range(G):
                    mm_group(m, g)
                mm_affine(m)
                matvec(m - 4, start=(m == 4), stop=False)
                act_sqrt(m)
                dve_tail(m)
            for m in range(4, M_TILES):
                matvec(m, start=False, stop=(m == M_TILES - 1))

            # local colsum -> SBUF -> AllGather
            nc.vector.tensor_copy(cs_sb, cps)
            nc.sync.dma_start(out=cc_in[:, :], in_=cs_sb)
            nc.gpsimd.collective_compute(
                "AllGather", A.bypass, replica_groups=groups,
                ins=[cc_in.opt()], outs=[cc_out.opt()],
            )

            # Q scale + store and squares run under the collective window
            for m in range(M_TILES):
                nc.vector.tensor_scalar_mul(q_all[:, m, :], qun_all[:, m, :],
                                            rq_all[:, m:m + 1])
                nc.sync.dma_start(
                    out=q_out.ap()[m * 128:(m + 1) * 128, :],
                    in_=q_all[:, m, :])
                nc.scalar.activation(q2_all[:, m, :], qun_all[:, m, :],
                                     F.Square)

            # gather -> sum+broadcast via one K=8 matmul -> 1/s (f32 + bf16)
            nc.sync.dma_start(out=ag_sb, in_=cc_out[:, :].bitcast(f32r))
            bps = cpsum.tile([128, NCL], f32, tag="bps")
            nc.tensor.matmul(bps, lhsT=ones8[:, :], rhs=ag_sb[:, :],
                             start=True, stop=True)
            nc.vector.reciprocal_approx_fast(out=rs_inv, in_=bps[:, :])

            # P phase: W on DVE (even tiles) + GpSimd (odd tiles) in
            # parallel; odd-tile scales on ACT, even-tile scales on DVE.
            for m in range(M_TILES):
                nc.vector.scalar_tensor_tensor(
                    out=w_all[:, m, :], in0=q2_all[:, m, :],
                    scalar=0.0, in1=rs_inv,
                    op0=A.bypass, op1=A.mult,
                    accum_out=ws_all[:, m:m + 1])
                nc.vector.reciprocal(rw_all[:, m:m + 1], ws_all[:, m:m + 1])
                pt = pout.tile([128, NCL], bf16, tag="pt")
                if m % 2 == 0:
                    nc.vector.tensor_scalar_mul(pt, w_all[:, m, :],
                                                rw_all[:, m:m + 1])
                    nc.sync.dma_start(
                        out=p_out.ap()[m * 128:(m + 1) * 128, :], in_=pt)
                else:
                    nc.scalar.activation(pt, w_all[:, m, :], F.Copy,
                                         scale=rw_all[:, m:m + 1])
                    nc.scalar.dma_start(
                        out=p_out.ap()[m * 128:(m + 1) * 128, :], in_=pt)
    nc.compile()
    return nc


def _get_nc():
    global _nc_cache
    if _nc_cache is None:
        _nc_cache = _build_nc()
    return _nc_cache


def _prep_inputs(z, centroids):
    z = np.asarray(z, dtype=np.float32)
    c = np.asarray(centroids, dtype=np.float32)

    # fp8 cross-term operands; contraction row h = 256g + 128ko + ki
    z8 = z.astype(F8)                                   # (8192, 2048)
    c8m2 = (-2.0 * c.astype(F8).astype(np.float32)).astype(F8)
    ct_full = np.ascontiguousarray(
        c8m2.T.reshape(G, 2, 128, NCL).transpose(0, 2, 1, 3)
    ).reshape(G, 128, 2 * NCL)

    csq = np.sum(c.astype(np.float64) ** 2, axis=1)     # (512,)
    affr = np.empty((2, NCL), dtype=np.float32)
    affr[0] = (csq - 1.0).astype(np.float32)
    affr[1] = 1.0

    zsq = np.sum(z.astype(np.float64) ** 2, axis=1)     # (8192,)

    in_maps = []
    for core in range(N_CORES):
        s = slice(core * B_CORE, (core + 1) * B_CORE)
        zt_core = np.ascontiguousarray(
            z8[s].T.reshape(G, 2, 128, B_CORE).transpose(0, 2, 1, 3)
        ).reshape(G, 128, 2 * B_CORE)
        affl = np.empty((2, B_CORE), dtype=np.float32)
        affl[0] = 1.0
        affl[1] = zsq[s].astype(np.float32)
        in_maps.append({"zt": zt_core, "ct": ct_full,
                        "affl": affl, "affr": affr,
                        "ones8": np.ones((8, 128), dtype=np.float32)})
    return in_maps


def run(z, centroids, trace=False, trace_cores=None):
    """Run on the 8 NeuronCores. Returns (out, BassKernelResults)."""
    nc = _get_nc()
    in_maps = _prep_inputs(z, centroids)
    res = run_bass_kernel_spmd(
        nc, in_maps, list(range(N_CORES)),
        trace=trace, trace_cores=trace_cores,
    )
    q = np.concatenate([np.asarray(res.results[c]["q"], dtype=np.float32)
                        for c in range(N_CORES)], axis=0)
    p = np.concatenate([np.asarray(res.results[c]["p"], dtype=np.float32)
                        for c in range(N_CORES)], axis=0)
    out = np.stack([q, p])
    return out, res


def kernel(z, centroids):
    out, _ = run(z, centroids)
    return out
